# revision 5
# baseline (speedup 1.0000x reference)
"""Causal self-attention (B=4, S=2048, D=1024, H=16) on 8 trn2 NeuronCores.

Sharding: core c handles batch b = c//2 and heads h0 = (c%2)*8
(data parallel over batch x tensor parallel over head halves).

bf16 data path (fp32 PSUM accumulation everywhere):
  Phase P: single pass over x (resident in SBUF, loaded once with
    full-row DMAs): Q^T (bias folded via DVE add), K^T (per head,
    zero-padded to 128 feature rows), V (per k-tile, with a ones
    column per head so PV row 64 yields softmax row-sums for free).
  Phase A: attention in two q-blocks of 1024 columns. Per (qb, head,
    kt): scores into a 2-bank PSUM tile (two <=512-col matmuls), ONE
    exp over <=1024 columns on ACT (minimizes per-instruction PSUM
    access overhead; ACT cost is per-column), multiplicative 0/1
    causal mask on the diagonal 128-block (DVE, bf16 2x mode), PV
    accumulation into a double-buffered [65,1024] PSUM tile.
    Normalization per (qb, head): reciprocal row-sums (DVE) ->
    partition broadcast (GPSIMD) -> scale to OT (DVE), overlapped
    with the next head via PSUM double buffering.
  Phase C: y_part = OT^T @ w_proj rows, streamed out per token tile.
Host: y[b] = part(core 2b) + part(core 2b+1) + b_proj + b_v @ w_proj.
"""

import numpy as np
import ml_dtypes

D_MODEL = 1024
N_HEADS = 16
HEAD_DIM = 64
B = 4
S = 2048
HPC = 8          # heads per core
CORES = 8
FPC = HPC * HEAD_DIM  # 512 features per core

_CACHE = {}


def _build():
    import concourse.bacc as bacc
    import concourse.tile as tile
    import concourse.mybir as mybir

    f32 = mybir.dt.float32
    bf16 = mybir.dt.bfloat16
    Exp = mybir.ActivationFunctionType.Exp

    nc = bacc.Bacc("TRN2", debug=False)
    # host pre-packs: xT [D, S] bf16; wqkv [D, 3*FPC] bf16 (q cols
    # pre-scaled by 1/sqrt(dh)); bqr [128, 4] f32; wp [FPC, D] bf16
    xT = nc.dram_tensor("xT", [D_MODEL, S], bf16, kind="ExternalInput").ap()
    wqkv = nc.dram_tensor("wqkv", [D_MODEL, 3 * FPC], bf16,
                          kind="ExternalInput").ap()
    bqr = nc.dram_tensor("bqr", [128, 4], f32, kind="ExternalInput").ap()
    wp = nc.dram_tensor("wp", [FPC, D_MODEL], bf16, kind="ExternalInput").ap()
    y = nc.dram_tensor("y", [S, D_MODEL], f32, kind="ExternalOutput").ap()

    NT = S // 128        # 16 token tiles
    KCH = D_MODEL // 128  # 8 contraction chunks

    with tile.TileContext(nc) as tc:
        with tc.tile_pool(name="persist", bufs=1) as persist:
            xsb = [persist.tile([128, S], bf16, name=f"xsb{k}")
                   for k in range(KCH)]
            wqkv_sb = [persist.tile([128, 3 * FPC], bf16, name=f"wqkvs{k}")
                       for k in range(KCH)]
            QT = [persist.tile([128, S], bf16, name=f"QT{p}") for p in range(4)]
            KTh = [persist.tile([128, S], bf16, name=f"KTh{h}")
                   for h in range(HPC)]
            Vb = [persist.tile([128, HPC * 65], bf16, name=f"Vb{k}")
                  for k in range(NT)]
            OT = [persist.tile([128, S], bf16, name=f"OT{p}") for p in range(4)]
            wp_sb = [persist.tile([128, D_MODEL], bf16, name=f"wps{p}")
                     for p in range(4)]
            bq_sb = persist.tile([128, 4], f32, name="bq_sb")
            tri32 = persist.tile([128, 128], f32, name="tri32")
            tri = persist.tile([128, 128], bf16, name="tri")

            # weights + bias first (small), then x full rows: all 4KB+
            # contiguous per-partition segments
            for k in range(KCH):
                nc.sync.dma_start(out=wqkv_sb[k],
                                  in_=wqkv[k * 128:(k + 1) * 128, :])
            nc.sync.dma_start(out=bq_sb, in_=bqr)
            for p in range(4):
                nc.sync.dma_start(out=wp_sb[p],
                                  in_=wp[p * 128:(p + 1) * 128, :])
            for k in range(KCH):
                nc.sync.dma_start(out=xsb[k],
                                  in_=xT[k * 128:(k + 1) * 128, :])

            # 0/1 multiplicative causal mask for the diagonal block:
            # tri[k, q] = 1 where q >= k else 0 (built f32, cast to bf16)
            nc.gpsimd.memset(tri32, 1.0)
            nc.gpsimd.affine_select(
                out=tri32, in_=tri32,
                compare_op=mybir.AluOpType.is_ge, fill=0.0,
                base=0, pattern=[[1, 128]], channel_multiplier=-1)
            nc.vector.tensor_copy(out=tri, in_=tri32)
            for kt in range(NT):
                ones_col = Vb[kt].rearrange("p (h d) -> p h d", d=65)[:, :, 64:65]
                nc.gpsimd.memset(ones_col, 1.0)
            for h in range(HPC):
                dead = (slice(64, 128) if h % 2 == 0 else slice(0, 64))
                nc.gpsimd.memset(KTh[h][dead, :], 0.0)

            # ---------------- Phase P: projections, one x pass ----------
            with tc.tile_pool(name="ppool", bufs=2, space="PSUM") as pp:
                for tch in range(4):
                    sl = slice(tch * 512, (tch + 1) * 512)
                    for m in range(4):
                        psq = pp.tile([128, 512], f32, name="psq", tag="psq")
                        for k in range(KCH):
                            nc.tensor.matmul(
                                psq,
                                lhsT=wqkv_sb[k][:, m * 128:(m + 1) * 128],
                                rhs=xsb[k][:, sl],
                                start=(k == 0), stop=(k == KCH - 1))
                        nc.vector.tensor_scalar_add(
                            QT[m][:, sl], psq, bq_sb[:, m:m + 1])
                        psk = pp.tile([128, 512], f32, name="psk", tag="psk")
                        for k in range(KCH):
                            nc.tensor.matmul(
                                psk,
                                lhsT=wqkv_sb[k][:, FPC + m * 128:FPC + (m + 1) * 128],
                                rhs=xsb[k][:, sl],
                                start=(k == 0), stop=(k == KCH - 1))
                        nc.scalar.copy(out=KTh[2 * m][0:64, sl],
                                       in_=psk[0:64, :])
                        nc.scalar.copy(out=KTh[2 * m + 1][64:128, sl],
                                       in_=psk[64:128, :])
                    for tt in range(4):
                        psv = pp.tile([128, 512], f32, name="psv", tag="psv")
                        for k in range(KCH):
                            nc.tensor.matmul(
                                psv,
                                lhsT=xsb[k][:, tch * 512 + tt * 128:
                                            tch * 512 + (tt + 1) * 128],
                                rhs=wqkv_sb[k][:, 2 * FPC:3 * FPC],
                                start=(k == 0), stop=(k == KCH - 1))
                        nc.vector.tensor_copy(
                            out=Vb[tch * 4 + tt].rearrange(
                                "p (h d) -> p h d", d=65)[:, :, 0:64],
                            in_=psv.rearrange("p (h d) -> p h d", d=64))

            # ---------------- Phase A: attention, two 1024-col q-blocks --
            with tc.tile_pool(name="apool", bufs=1, space="PSUM") as ap, \
                 tc.tile_pool(name="ptpool", bufs=3) as ptpool, \
                 tc.tile_pool(name="work", bufs=2) as work:
                for qb in range(2):
                    q_lo, q_hi = qb * 1024, (qb + 1) * 1024
                    nkt = (q_hi // 128)
                    for h in range(HPC):
                        p_idx, part = h // 2, (h % 2) * 64
                        pso = ap.tile([65, 1024], f32, name="pso",
                                      tag=f"pso{(h + qb * 8) % 2}")
                        for kt in range(nkt):
                            k0 = kt * 128
                            c_lo = max(k0, q_lo)  # first col this kt touches
                            w = q_hi - c_lo
                            pss = ap.tile([128, 1024], f32, name="pss",
                                          tag=f"pss{kt % 2}")
                            off = c_lo - q_lo
                            o = 0
                            while o < w:
                                sz = min(512 - (off + o) % 512, w - o)
                                nc.tensor.matmul(
                                    pss[:, off + o:off + o + sz],
                                    lhsT=KTh[h][:, k0:k0 + 128],
                                    rhs=QT[p_idx][:, c_lo + o:c_lo + o + sz],
                                    start=True, stop=True)
                                o += sz
                            pt = ptpool.tile([128, 1024], bf16, name="pt",
                                             tag="pt")
                            nc.scalar.activation(
                                out=pt[:, 0:w], in_=pss[:, off:off + w],
                                func=Exp)
                            if c_lo == k0:
                                dw = min(128, w)
                                nc.vector.tensor_mul(
                                    pt[:, 0:dw], pt[:, 0:dw], tri[:, 0:dw])
                            o = 0
                            while o < w:
                                sz = min(512 - (off + o) % 512, w - o)
                                nc.tensor.matmul(
                                    pso[:, off + o:off + o + sz],
                                    lhsT=Vb[kt][:, h * 65:h * 65 + 65],
                                    rhs=pt[:, o:o + sz],
                                    start=(kt == 0), stop=(kt == nkt - 1),
                                    skip_group_check=True)
                                o += sz
                        # normalize: per-column 1/rowsum, rowsums in row 64
                        rs0 = work.tile([1, 1024], f32, name="rs0", tag="rs0")
                        nc.vector.tensor_copy(rs0, pso[64:65, :])
                        rs = work.tile([1, 1024], f32, name="rs", tag="rs")
                        nc.vector.reciprocal_approx_fast(out=rs, in_=rs0)
                        rc = work.tile([64, 1024], f32, name="rc", tag="rc")
                        nc.gpsimd.partition_broadcast(rc, rs[0:1, :])
                        if part == 0:
                            nc.vector.tensor_mul(
                                OT[p_idx][0:64, q_lo:q_hi], pso[0:64, :], rc)
                        else:
                            tmp = work.tile([64, 1024], bf16, name="tmp",
                                            tag="tmp")
                            nc.vector.tensor_mul(tmp, pso[0:64, :], rc)
                            nc.sync.dma_start(
                                out=OT[p_idx][64:128, q_lo:q_hi], in_=tmp)

            # ---------------- Phase C: output projection ----------------
            with tc.tile_pool(name="cpool", bufs=2, space="PSUM") as cp, \
                 tc.tile_pool(name="ypool", bufs=3) as ypool:
                for tt in range(NT):
                    psy = cp.tile([128, 1024], f32, name="psy", tag="psy")
                    for half in range(2):
                        for p in range(4):
                            nc.tensor.matmul(
                                psy[:, half * 512:(half + 1) * 512],
                                lhsT=OT[p][:, tt * 128:(tt + 1) * 128],
                                rhs=wp_sb[p][:, half * 512:(half + 1) * 512],
                                start=(p == 0), stop=(p == 3))
                    ysb = ypool.tile([128, 1024], f32, name="ysb", tag="ysb")
                    nc.scalar.copy(out=ysb, in_=psy)
                    nc.sync.dma_start(
                        out=y[tt * 128:(tt + 1) * 128, :], in_=ysb)

    nc.compile()
    return nc


def _get_nc():
    if "nc" not in _CACHE:
        _CACHE["nc"] = _build()
    return _CACHE["nc"]


def _make_in_maps(x, w_qkv, b_qkv, w_proj):
    bf = ml_dtypes.bfloat16
    scale = np.float32(1.0 / np.sqrt(HEAD_DIM))
    in_maps = []
    for c in range(CORES):
        b = c // 2
        h0 = (c % 2) * HPC
        f0 = h0 * HEAD_DIM
        wq = w_qkv[:, f0:f0 + FPC] * scale
        wk = w_qkv[:, D_MODEL + f0:D_MODEL + f0 + FPC]
        wv = w_qkv[:, 2 * D_MODEL + f0:2 * D_MODEL + f0 + FPC]
        in_maps.append({
            "xT": np.ascontiguousarray(x[b].T).astype(bf),
            "wqkv": np.ascontiguousarray(
                np.concatenate([wq, wk, wv], axis=1)).astype(bf),
            "bqr": np.ascontiguousarray(
                (b_qkv[f0:f0 + FPC] * scale).reshape(4, 128).T
            ).astype(np.float32),
            "wp": np.ascontiguousarray(w_proj[f0:f0 + FPC, :]).astype(bf),
        })
    return in_maps


def kernel(x, w_qkv, b_qkv, w_proj, b_proj):
    from concourse.bass_utils import run_bass_kernel_spmd

    x = np.asarray(x, np.float32)
    w_qkv = np.asarray(w_qkv, np.float32)
    b_qkv = np.asarray(b_qkv, np.float32)
    w_proj = np.asarray(w_proj, np.float32)
    b_proj = np.asarray(b_proj, np.float32)

    nc = _get_nc()
    in_maps = _make_in_maps(x, w_qkv, b_qkv, w_proj)
    res = run_bass_kernel_spmd(nc, in_maps, core_ids=list(range(CORES)))

    # host-side bias corrections: b_proj plus b_v routed through w_proj
    # (softmax rows sum to one, so P @ (1 b_v^T) W_p = 1 (b_v^T W_p))
    bv = b_qkv[2 * D_MODEL:3 * D_MODEL]
    bias_row = (bv @ w_proj + b_proj).astype(np.float32)

    out = np.empty((B, S, D_MODEL), np.float32)
    for b in range(B):
        out[b] = res.results[2 * b]["y"] + res.results[2 * b + 1]["y"] + bias_row
    return out


# revision 10
# speedup vs baseline: 1.0169x; 1.0169x over previous
"""Causal self-attention (B=4, S=2048, D=1024, H=16) on 8 trn2 NeuronCores.

Sharding: core c handles batch b = c//2 and heads h0 = (c%2)*8
(data parallel over batch x tensor parallel over head halves).

bf16 data path (fp32 PSUM accumulation everywhere):
  Phase P: single pass over x (resident in SBUF, loaded once with
    full-row DMAs): Q^T (bias folded via DVE add), K^T (per head,
    zero-padded to 128 feature rows), V (per k-tile, with a ones
    column per head so PV row 64 yields softmax row-sums for free).
  Phase A: attention in two q-blocks of 1024 columns. Per (qb, head,
    kt): scores into a 2-bank PSUM tile (two <=512-col matmuls), ONE
    exp over <=1024 columns on ACT (minimizes per-instruction PSUM
    access overhead; ACT cost is per-column), multiplicative 0/1
    causal mask on the diagonal 128-block (DVE, bf16 2x mode), PV
    accumulation into a double-buffered [65,1024] PSUM tile.
    Normalization per (qb, head): reciprocal row-sums (DVE) ->
    partition broadcast (GPSIMD) -> scale to OT (DVE), overlapped
    with the next head via PSUM double buffering.
  Phase C: y_part = OT^T @ w_proj rows, streamed out per token tile.
Host: y[b] = part(core 2b) + part(core 2b+1) + b_proj + b_v @ w_proj.
"""

import numpy as np
import ml_dtypes

D_MODEL = 1024
N_HEADS = 16
HEAD_DIM = 64
B = 4
S = 2048
HPC = 8          # heads per core
CORES = 8
FPC = HPC * HEAD_DIM  # 512 features per core

_CACHE = {}


def _build():
    import concourse.bacc as bacc
    import concourse.tile as tile
    import concourse.mybir as mybir

    f32 = mybir.dt.float32
    bf16 = mybir.dt.bfloat16
    Exp = mybir.ActivationFunctionType.Exp

    nc = bacc.Bacc("TRN2", debug=False)
    # host pre-packs: xT [D, S] bf16; wqkv [D, 3*FPC] bf16 (q cols
    # pre-scaled by 1/sqrt(dh)); bqr [128, 4] f32; wp [FPC, D] bf16
    xT = nc.dram_tensor("xT", [D_MODEL, S], bf16, kind="ExternalInput").ap()
    wqkv = nc.dram_tensor("wqkv", [D_MODEL, 3 * FPC], bf16,
                          kind="ExternalInput").ap()
    bqr = nc.dram_tensor("bqr", [128, 4], f32, kind="ExternalInput").ap()
    wp = nc.dram_tensor("wp", [FPC, D_MODEL], bf16, kind="ExternalInput").ap()
    y = nc.dram_tensor("y", [S, D_MODEL], f32, kind="ExternalOutput").ap()

    NT = S // 128        # 16 token tiles
    KCH = D_MODEL // 128  # 8 contraction chunks

    with tile.TileContext(nc) as tc:
        with tc.tile_pool(name="persist", bufs=1) as persist:
            xsb = [persist.tile([128, S], bf16, name=f"xsb{k}")
                   for k in range(KCH)]
            wqkv_sb = [persist.tile([128, 3 * FPC], bf16, name=f"wqkvs{k}")
                       for k in range(KCH)]
            QT = [persist.tile([128, S], bf16, name=f"QT{p}") for p in range(4)]
            KTh = [persist.tile([128, S], bf16, name=f"KTh{h}")
                   for h in range(HPC)]
            Vb = [persist.tile([128, HPC * 65], bf16, name=f"Vb{k}")
                  for k in range(NT)]
            OT = [persist.tile([128, S], bf16, name=f"OT{p}") for p in range(4)]
            wp_sb = [persist.tile([128, D_MODEL], bf16, name=f"wps{p}")
                     for p in range(4)]
            bq_sb = persist.tile([128, 4], f32, name="bq_sb")
            tri32 = persist.tile([128, 128], f32, name="tri32")
            tri = persist.tile([128, 128], bf16, name="tri")

            # x + qkv weights first (phase P critical path); wp only
            # needed in phase C so it loads last. All rows are 3KB+
            # contiguous per-partition segments.
            for k in range(KCH):
                nc.sync.dma_start(out=xsb[k],
                                  in_=xT[k * 128:(k + 1) * 128, :])
                nc.sync.dma_start(out=wqkv_sb[k],
                                  in_=wqkv[k * 128:(k + 1) * 128, :])
            nc.sync.dma_start(out=bq_sb, in_=bqr)
            for p in range(4):
                nc.sync.dma_start(out=wp_sb[p],
                                  in_=wp[p * 128:(p + 1) * 128, :])

            # 0/1 multiplicative causal mask for the diagonal block:
            # tri[k, q] = 1 where q >= k else 0 (built f32, cast to bf16)
            nc.gpsimd.memset(tri32, 1.0)
            nc.gpsimd.affine_select(
                out=tri32, in_=tri32,
                compare_op=mybir.AluOpType.is_ge, fill=0.0,
                base=0, pattern=[[1, 128]], channel_multiplier=-1)
            nc.vector.tensor_copy(out=tri, in_=tri32)
            for kt in range(NT):
                ones_col = Vb[kt].rearrange("p (h d) -> p h d", d=65)[:, :, 64:65]
                nc.gpsimd.memset(ones_col, 1.0)
            for h in range(HPC):
                dead = (slice(64, 128) if h % 2 == 0 else slice(0, 64))
                nc.gpsimd.memset(KTh[h][dead, :], 0.0)

            # ---------------- Phase P: projections, one x pass ----------
            with tc.tile_pool(name="ppool", bufs=2, space="PSUM") as pp:
                for tch in range(4):
                    sl = slice(tch * 512, (tch + 1) * 512)
                    for m in range(4):
                        psq = pp.tile([128, 512], f32, name="psq", tag="psq")
                        for k in range(KCH):
                            nc.tensor.matmul(
                                psq,
                                lhsT=wqkv_sb[k][:, m * 128:(m + 1) * 128],
                                rhs=xsb[k][:, sl],
                                start=(k == 0), stop=(k == KCH - 1))
                        nc.vector.tensor_scalar_add(
                            QT[m][:, sl], psq, bq_sb[:, m:m + 1])
                        psk = pp.tile([128, 512], f32, name="psk", tag="psk")
                        for k in range(KCH):
                            nc.tensor.matmul(
                                psk,
                                lhsT=wqkv_sb[k][:, FPC + m * 128:FPC + (m + 1) * 128],
                                rhs=xsb[k][:, sl],
                                start=(k == 0), stop=(k == KCH - 1))
                        nc.scalar.copy(out=KTh[2 * m][0:64, sl],
                                       in_=psk[0:64, :])
                        nc.scalar.copy(out=KTh[2 * m + 1][64:128, sl],
                                       in_=psk[64:128, :])
                    for tt in range(4):
                        psv = pp.tile([128, 512], f32, name="psv", tag="psv")
                        for k in range(KCH):
                            nc.tensor.matmul(
                                psv,
                                lhsT=xsb[k][:, tch * 512 + tt * 128:
                                            tch * 512 + (tt + 1) * 128],
                                rhs=wqkv_sb[k][:, 2 * FPC:3 * FPC],
                                start=(k == 0), stop=(k == KCH - 1))
                        nc.vector.tensor_copy(
                            out=Vb[tch * 4 + tt].rearrange(
                                "p (h d) -> p h d", d=65)[:, :, 0:64],
                            in_=psv.rearrange("p (h d) -> p h d", d=64))

            # ---------------- Phase A: attention, two 1024-col q-blocks --
            with tc.tile_pool(name="apool", bufs=1, space="PSUM") as ap, \
                 tc.tile_pool(name="ptpool", bufs=4) as ptpool, \
                 tc.tile_pool(name="work", bufs=2) as work:
                for qb in range(2):
                    q_lo, q_hi = qb * 1024, (qb + 1) * 1024
                    nkt = (q_hi // 128)
                    # odd heads (whose OT writes go via DMA) run first so
                    # the final OT dependency of phase C is a cheap DVE op
                    for hi, h in enumerate((1, 0, 3, 2, 5, 4, 7, 6)):
                        p_idx, part = h // 2, (h % 2) * 64
                        pso = ap.tile([65, 1024], f32, name="pso",
                                      tag=f"pso{hi % 2}")
                        for kt in range(nkt):
                            k0 = kt * 128
                            c_lo = max(k0, q_lo)  # first col this kt touches
                            w = q_hi - c_lo
                            pss = ap.tile([128, 1024], f32, name="pss",
                                          tag=f"pss{kt % 2}")
                            off = c_lo - q_lo
                            o = 0
                            while o < w:
                                sz = min(512 - (off + o) % 512, w - o)
                                nc.tensor.matmul(
                                    pss[:, off + o:off + o + sz],
                                    lhsT=KTh[h][:, k0:k0 + 128],
                                    rhs=QT[p_idx][:, c_lo + o:c_lo + o + sz],
                                    start=True, stop=True)
                                o += sz
                            pt = ptpool.tile([128, 1024], bf16, name="pt",
                                             tag="pt")
                            nc.scalar.activation(
                                out=pt[:, 0:w], in_=pss[:, off:off + w],
                                func=Exp)
                            if c_lo == k0:
                                dw = min(128, w)
                                nc.vector.tensor_mul(
                                    pt[:, 0:dw], pt[:, 0:dw], tri[:, 0:dw])
                            # emit PV sub-chunks high-to-low: the diagonal
                            # (masked) columns run last, hiding the DVE
                            # mask latency behind the unmasked matmul
                            subs = []
                            o = 0
                            while o < w:
                                sz = min(512 - (off + o) % 512, w - o)
                                subs.append((o, sz))
                                o += sz
                            for o, sz in reversed(subs):
                                nc.tensor.matmul(
                                    pso[:, off + o:off + o + sz],
                                    lhsT=Vb[kt][:, h * 65:h * 65 + 65],
                                    rhs=pt[:, o:o + sz],
                                    start=(kt == 0), stop=(kt == nkt - 1),
                                    skip_group_check=True)
                        # normalize: per-column 1/rowsum, rowsums in row 64
                        rs0 = work.tile([1, 1024], f32, name="rs0", tag="rs0")
                        nc.vector.tensor_copy(rs0, pso[64:65, :])
                        rs = work.tile([1, 1024], f32, name="rs", tag="rs")
                        nc.vector.reciprocal_approx_fast(out=rs, in_=rs0)
                        rc = work.tile([64, 1024], f32, name="rc", tag="rc")
                        nc.gpsimd.partition_broadcast(rc, rs[0:1, :])
                        if part == 0:
                            nc.vector.tensor_mul(
                                OT[p_idx][0:64, q_lo:q_hi], pso[0:64, :], rc)
                        else:
                            tmp = work.tile([64, 1024], bf16, name="tmp",
                                            tag="tmp")
                            nc.vector.tensor_mul(tmp, pso[0:64, :], rc)
                            nc.sync.dma_start(
                                out=OT[p_idx][64:128, q_lo:q_hi], in_=tmp)

            # ---------------- Phase C: output projection ----------------
            with tc.tile_pool(name="cpool", bufs=2, space="PSUM") as cp, \
                 tc.tile_pool(name="ypool", bufs=3) as ypool:
                for tt in range(NT):
                    psy = cp.tile([128, 1024], f32, name="psy", tag="psy")
                    for half in range(2):
                        for p in range(4):
                            nc.tensor.matmul(
                                psy[:, half * 512:(half + 1) * 512],
                                lhsT=OT[p][:, tt * 128:(tt + 1) * 128],
                                rhs=wp_sb[p][:, half * 512:(half + 1) * 512],
                                start=(p == 0), stop=(p == 3))
                    ysb = ypool.tile([128, 1024], f32, name="ysb", tag="ysb")
                    nc.scalar.copy(out=ysb, in_=psy)
                    nc.sync.dma_start(
                        out=y[tt * 128:(tt + 1) * 128, :], in_=ysb)

    nc.compile()
    return nc


def _get_nc():
    if "nc" not in _CACHE:
        _CACHE["nc"] = _build()
    return _CACHE["nc"]


def _make_in_maps(x, w_qkv, b_qkv, w_proj):
    bf = ml_dtypes.bfloat16
    scale = np.float32(1.0 / np.sqrt(HEAD_DIM))
    in_maps = []
    for c in range(CORES):
        b = c // 2
        h0 = (c % 2) * HPC
        f0 = h0 * HEAD_DIM
        wq = w_qkv[:, f0:f0 + FPC] * scale
        wk = w_qkv[:, D_MODEL + f0:D_MODEL + f0 + FPC]
        wv = w_qkv[:, 2 * D_MODEL + f0:2 * D_MODEL + f0 + FPC]
        in_maps.append({
            "xT": np.ascontiguousarray(x[b].T).astype(bf),
            "wqkv": np.ascontiguousarray(
                np.concatenate([wq, wk, wv], axis=1)).astype(bf),
            "bqr": np.ascontiguousarray(
                (b_qkv[f0:f0 + FPC] * scale).reshape(4, 128).T
            ).astype(np.float32),
            "wp": np.ascontiguousarray(w_proj[f0:f0 + FPC, :]).astype(bf),
        })
    return in_maps


def kernel(x, w_qkv, b_qkv, w_proj, b_proj):
    from concourse.bass_utils import run_bass_kernel_spmd

    x = np.asarray(x, np.float32)
    w_qkv = np.asarray(w_qkv, np.float32)
    b_qkv = np.asarray(b_qkv, np.float32)
    w_proj = np.asarray(w_proj, np.float32)
    b_proj = np.asarray(b_proj, np.float32)

    nc = _get_nc()
    in_maps = _make_in_maps(x, w_qkv, b_qkv, w_proj)
    res = run_bass_kernel_spmd(nc, in_maps, core_ids=list(range(CORES)))

    # host-side bias corrections: b_proj plus b_v routed through w_proj
    # (softmax rows sum to one, so P @ (1 b_v^T) W_p = 1 (b_v^T W_p))
    bv = b_qkv[2 * D_MODEL:3 * D_MODEL]
    bias_row = (bv @ w_proj + b_proj).astype(np.float32)

    out = np.empty((B, S, D_MODEL), np.float32)
    for b in range(B):
        out[b] = res.results[2 * b]["y"] + res.results[2 * b + 1]["y"] + bias_row
    return out
